# revision 1
# baseline (speedup 1.0000x reference)
import sys

sys.path.insert(0, "/opt/trn_rl_repo")

import numpy as np
import ml_dtypes

# Phi3SeerAttention, B=1 S=2048 HIDDEN=3072, H=32 q heads, HK=8 kv heads,
# D=96, gate block 64, gate hidden 128. Sharded TP over kv heads: core c
# owns kv head c and q heads 4c..4c+3; o-proj row-sharded, partials summed
# on host (the gather step).
H, HK, D, BLK, GH = 32, 8, 96, 64, 128
S, HIDDEN = 2048, 3072
G = H // HK          # 4 q heads per kv head (per core)
NB = S // BLK        # 32 gate blocks
KT = HIDDEN // 128   # 24 contraction tiles
NS = S // 512        # 4 sequence chunks of 512
NT = S // 128        # 16 t-tiles of 128
NE = HIDDEN // 512   # 6 output column chunks
NCORES = 8
THR = 0.03

_prog = None


def _build(debug=False):
    from concourse import bass, mybir, bacc
    import concourse.tile as tile
    from contextlib import ExitStack

    dt = mybir.dt
    BF, F32 = dt.bfloat16, dt.float32
    AF = mybir.ActivationFunctionType
    OP = mybir.AluOpType
    AX = mybir.AxisListType.X

    nc = bacc.Bacc()
    xt_d = nc.dram_tensor("xt", [HIDDEN, S], BF, kind="ExternalInput")
    wq_d = nc.dram_tensor("wq", [HIDDEN, G * D], BF, kind="ExternalInput")
    wk_d = nc.dram_tensor("wk", [HIDDEN, D], BF, kind="ExternalInput")
    wv_d = nc.dram_tensor("wv", [HIDDEN, D], BF, kind="ExternalInput")
    ow_d = nc.dram_tensor("ow", [G * D, HIDDEN], BF, kind="ExternalInput")
    cosq_d = nc.dram_tensor("cosq", [D, S], BF, kind="ExternalInput")
    sinq_d = nc.dram_tensor("sinq", [D, S], BF, kind="ExternalInput")
    cosk_d = nc.dram_tensor("cosk", [D, S], BF, kind="ExternalInput")
    sink_d = nc.dram_tensor("sink", [D, S], BF, kind="ExternalInput")
    rot_d = nc.dram_tensor("rot", [D, D], BF, kind="ExternalInput")
    gwq_d = nc.dram_tensor("gwq", [D, GH], F32, kind="ExternalInput")
    gwk_d = nc.dram_tensor("gwk", [2 * D, GH], F32, kind="ExternalInput")
    eye_d = nc.dram_tensor("eye32", [NB, NB], F32, kind="ExternalInput")
    emat_d = nc.dram_tensor("emat", [NB, NT * 128], F32, kind="ExternalInput")
    bcm_d = nc.dram_tensor("bcm", [NB, NB], F32, kind="ExternalInput")
    cmask_d = nc.dram_tensor("cmask", [128, 4 * 512], BF, kind="ExternalInput")
    out_d = nc.dram_tensor("out_p", [S, HIDDEN], BF, kind="ExternalOutput")

    # Raw (persistent) SBUF tensors that cross the phase-1 barrier. The two
    # TileContexts are separated by a full drain+barrier so no instruction
    # ever needs to wait on the union of all 8 DMA HW queue semaphores
    # (compute-engine instructions have a small embedded sync-wait cap).
    q_sb = nc.alloc_sbuf_tensor("q_sbuf", [D, G, S], BF)
    k_sb = nc.alloc_sbuf_tensor("k_sbuf", [D, S], BF)
    v_sb = nc.alloc_sbuf_tensor("v_sbuf", [128, NT, D + 1], BF)
    qp_sb = nc.alloc_sbuf_tensor("qp_sbuf", [D, G, NB], F32)
    km_sb = nc.alloc_sbuf_tensor("km_sbuf", [D, NB], F32)
    kx_sb = nc.alloc_sbuf_tensor("kx_sbuf", [D, NB], F32)

    # ---- context A / phase 1: QKV projection + gate pooling ----
    with tile.TileContext(nc) as tc:
        with tc.tile_pool(name="xw", bufs=1) as xw, tc.tile_pool(
            name="ps1", bufs=6, space="PSUM"
        ) as ps1:
            xt_sb = xw.tile([128, KT, S], BF)
            wq_sb = xw.tile([128, KT, G * D], BF)
            wk_sb = xw.tile([128, KT, D], BF)
            wv_sb = xw.tile([128, KT, D], BF)
            for kt in range(KT):
                r = slice(kt * 128, (kt + 1) * 128)
                nc.sync.dma_start(wq_sb[:, kt, :], wq_d[r, :])
                nc.sync.dma_start(wk_sb[:, kt, :], wk_d[r, :])
                nc.sync.dma_start(wv_sb[:, kt, :], wv_d[r, :])
            nc.vector.memset(v_sb[:, :, D : D + 1], 1.0)

            for j in range(NS):
                sl = slice(j * 512, (j + 1) * 512)
                for kt in range(KT):
                    r = slice(kt * 128, (kt + 1) * 128)
                    nc.sync.dma_start(xt_sb[:, kt, sl], xt_d[r, sl])

                for hh in range(G + 1):  # 0..3 = q heads, 4 = k
                    ps = ps1.tile([128, 512], F32)
                    pq = ps[:D, :]
                    for kt in range(KT):
                        lhsT = (
                            wq_sb[:, kt, hh * D : (hh + 1) * D]
                            if hh < G
                            else wk_sb[:, kt, :]
                        )
                        nc.tensor.matmul(
                            pq,
                            lhsT,
                            xt_sb[:, kt, sl],
                            start=(kt == 0),
                            stop=(kt == KT - 1),
                        )
                    pr = pq.rearrange("p (b w) -> p b w", w=BLK)
                    bs = slice(j * 8, (j + 1) * 8)
                    if hh < G:
                        # block SUM; 1/BLK folded into gate scale
                        nc.vector.tensor_reduce(
                            qp_sb[:, hh, bs], pr, axis=AX, op=OP.add
                        )
                        nc.scalar.copy(q_sb[:, hh, sl], pq)
                    else:
                        # block SUM; 1/BLK folded into gwk rows on host
                        nc.vector.tensor_reduce(km_sb[:, bs], pr, axis=AX, op=OP.add)
                        nc.vector.tensor_reduce(kx_sb[:, bs], pr, axis=AX, op=OP.max)
                        nc.scalar.copy(k_sb[:, sl], pq)

                for ti in range(4 * j, 4 * (j + 1)):
                    ps = ps1.tile([128, 512], F32)
                    pv = ps[:, :D]
                    for kt in range(KT):
                        nc.tensor.matmul(
                            pv,
                            xt_sb[:, kt, ti * 128 : (ti + 1) * 128],
                            wv_sb[:, kt, :],
                            start=(kt == 0),
                            stop=(kt == KT - 1),
                        )
                    nc.scalar.copy(v_sb[:, ti, :D], pv)

    # ---- context B: gate, RoPE, attention, o-projection ----
    with tile.TileContext(nc) as tc:
        with ExitStack() as ctx:
            perm = ctx.enter_context(tc.tile_pool(name="perm", bufs=1))
            mask_sb = perm.tile([128, NT, NB], BF)
            rot_sb = perm.tile([D, D], BF)
            gwq_sb = perm.tile([D, GH], F32)
            gwk_sb = perm.tile([D, 2, GH], F32)
            eye_sb = perm.tile([NB, NB], F32)
            bcm_sb = perm.tile([NB, NB], F32)
            ones_sb = perm.tile([1, 128], BF)
            attn_sb = perm.tile([D, G, S], BF)  # normalized attn output^T
            cosq_sb = perm.tile([D, S], BF)
            sinq_sb = perm.tile([D, S], BF)
            cosk_sb = perm.tile([D, S], BF)
            sink_sb = perm.tile([D, S], BF)
            emat_sb = perm.tile([NB, NT * 128], F32)
            cmask_sb = perm.tile([128, 4 * 512], BF)
            ow_sb = perm.tile([D, G, HIDDEN], BF)

            nc.sync.dma_start(rot_sb[:], rot_d[:])
            nc.sync.dma_start(gwq_sb[:], gwq_d[:])
            nc.sync.dma_start(gwk_sb[:, 0, :], gwk_d[0:D, :])
            nc.sync.dma_start(gwk_sb[:, 1, :], gwk_d[D : 2 * D, :])
            nc.sync.dma_start(eye_sb[:], eye_d[:])
            nc.sync.dma_start(bcm_sb[:], bcm_d[:])
            nc.sync.dma_start(cosq_sb[:], cosq_d[:])
            nc.sync.dma_start(sinq_sb[:], sinq_d[:])
            nc.sync.dma_start(cosk_sb[:], cosk_d[:])
            nc.sync.dma_start(sink_sb[:], sink_d[:])
            nc.sync.dma_start(emat_sb[:], emat_d[:])
            nc.sync.dma_start(cmask_sb[:], cmask_d[:])
            for hh in range(G):
                nc.sync.dma_start(ow_sb[:, hh, :], ow_d[hh * D : (hh + 1) * D, :])
            nc.vector.memset(ones_sb[:], 1.0)

            # ---- phase 2: block gate (fp32) ----
            with tc.tile_pool(name="gp", bufs=1) as gp, tc.tile_pool(
                name="gps", bufs=1, space="PSUM"
            ) as gps, tc.tile_pool(name="gpsm", bufs=2, space="PSUM") as gpsm:
                t0 = gp.tile([D, NB], F32)
                qps = gp.tile([D, NB], F32)
                nc.vector.tensor_add(t0[:], qp_sb[:, 0, :], qp_sb[:, 1, :])
                nc.vector.tensor_add(qps[:], qp_sb[:, 2, :], qp_sb[:, 3, :])
                nc.vector.tensor_add(qps[:], t0[:], qps[:])

                kg_ps = gps.tile([NB, GH], F32)
                nc.tensor.matmul(kg_ps, km_sb[:], gwk_sb[:, 0, :], start=True, stop=False)
                nc.tensor.matmul(kg_ps, kx_sb[:], gwk_sb[:, 1, :], start=False, stop=True)
                qg_ps = gps.tile([NB, GH], F32)
                nc.tensor.matmul(qg_ps, qps[:], gwq_sb[:], start=True, stop=True)
                qg_sb = gp.tile([NB, GH], F32)
                kg_sb = gp.tile([NB, GH], F32)
                # fold mean-over-heads (1/G), block mean (1/BLK), GH^-0.5
                nc.scalar.mul(qg_sb[:], qg_ps[:], (1.0 / (G * BLK)) * GH**-0.5)
                nc.scalar.copy(kg_sb[:], kg_ps[:])

                qgT_ps = gps.tile([GH, NB], F32)
                nc.tensor.matmul(qgT_ps, qg_sb[:], eye_sb[:], start=True, stop=True)
                kgT_ps = gps.tile([GH, NB], F32)
                nc.tensor.matmul(kgT_ps, kg_sb[:], eye_sb[:], start=True, stop=True)
                qgT_sb = gp.tile([GH, NB], F32)
                kgT_sb = gp.tile([GH, NB], F32)
                nc.scalar.copy(qgT_sb[:], qgT_ps[:])
                nc.scalar.copy(kgT_sb[:], kgT_ps[:])

                lg_ps = gps.tile([NB, NB], F32)
                nc.tensor.matmul(lg_ps, qgT_sb[:], kgT_sb[:], start=True, stop=True)
                lg_sb = gp.tile([NB, NB], F32)
                nc.scalar.copy(lg_sb[:], lg_ps[:])
                lm_sb = gp.tile([NB, NB], F32)
                nc.vector.tensor_add(lm_sb[:], lg_sb[:], bcm_sb[:])
                ge_sb = gp.tile([NB, NB], F32)
                gsum = gp.tile([NB, 1], F32)
                nc.scalar.activation(ge_sb[:], lm_sb[:], AF.Exp, accum_out=gsum[:])
                grc = gp.tile([NB, 1], F32)
                nc.vector.reciprocal(grc[:], gsum[:])
                prob_sb = gp.tile([NB, NB], F32)
                nc.scalar.activation(prob_sb[:], ge_sb[:], AF.Copy, scale=grc[:])
                m01 = gp.tile([NB, NB], F32)
                nc.vector.tensor_scalar(m01[:], prob_sb[:], THR, None, op0=OP.is_ge)
                nc.vector.tensor_tensor(m01[:], m01[:], eye_sb[:], op=OP.max)
                # transpose: expansion partitions index k blocks, m01 rows
                # index q blocks
                m01t_ps = gps.tile([NB, NB], F32)
                nc.tensor.matmul(m01t_ps, m01[:], eye_sb[:], start=True, stop=True)
                m01t = gp.tile([NB, NB], F32)
                nc.scalar.copy(m01t[:], m01t_ps[:])

                if debug:
                    for nm, t in [
                        ("dlg", lg_sb),
                        ("dqg", qg_sb),
                        ("dkg", kg_sb),
                        ("dprob", prob_sb),
                        ("dm01", m01),
                    ]:
                        dd = nc.dram_tensor(
                            nm, list(t[:].shape), t[:].dtype, kind="ExternalOutput"
                        )
                        nc.sync.dma_start(dd[:], t[:])

                for i in range(NT):
                    mp = gpsm.tile([128, NB], F32)
                    nc.tensor.matmul(
                        mp,
                        emat_sb[:, i * 128 : (i + 1) * 128],
                        m01t[:],
                        start=True,
                        stop=True,
                    )
                    nc.scalar.copy(mask_sb[:, i, :], mp[:])

            # ---- phase 3: RoPE in place on q^T / k^T ----
            with tc.tile_pool(name="rp", bufs=4) as rp, tc.tile_pool(
                name="rps", bufs=4, space="PSUM"
            ) as rps:
                for hh in range(G + 1):
                    src = q_sb[:, hh, :] if hh < G else k_sb[:]
                    cs = cosq_sb if hh < G else cosk_sb
                    sn = sinq_sb if hh < G else sink_sb
                    for j in range(NS):
                        sl = slice(j * 512, (j + 1) * 512)
                        rt = rps.tile([D, 512], F32)
                        nc.tensor.matmul(rt, rot_sb[:], src[:, sl], start=True, stop=True)
                        t1 = rp.tile([D, 512], BF)
                        nc.vector.tensor_mul(t1[:], src[:, sl], cs[:, sl])
                        t2 = rp.tile([D, 512], BF)
                        nc.vector.tensor_mul(t2[:], rt[:], sn[:, sl])
                        nc.vector.tensor_add(src[:, sl], t1[:], t2[:])

            # ---- phase 4: masked attention (transposed P layout) ----
            from concourse.bass import AP

            with tc.tile_pool(name="ap_", bufs=4) as ap_, tc.tile_pool(
                name="sm", bufs=4
            ) as sm, tc.tile_pool(name="sps", bufs=3, space="PSUM") as sps, tc.tile_pool(
                name="pvs", bufs=2, space="PSUM"
            ) as pvs, tc.tile_pool(name="rbs", bufs=2, space="PSUM") as rbs:
                for hh in range(G):
                    for j in range(NS):
                        ssl = slice(j * 512, (j + 1) * 512)
                        pv_ps = pvs.tile([D + 1, 512], F32)
                        ntile = 4 * (j + 1)
                        for ti in range(ntile):
                            s_ps = sps.tile([128, 512], F32)
                            nc.tensor.matmul(
                                s_ps,
                                k_sb[:, ti * 128 : (ti + 1) * 128],
                                q_sb[:, hh, ssl],
                                start=True,
                                stop=True,
                                skip_group_check=True,
                            )
                            p_sb = ap_.tile([128, 512], BF)
                            nc.scalar.activation(p_sb[:], s_ps[:], AF.Exp)
                            if ti >= 4 * j:
                                r = ti - 4 * j
                                nc.vector.tensor_mul(
                                    p_sb[:],
                                    p_sb[:],
                                    cmask_sb[:, r * 512 : (r + 1) * 512],
                                )
                            msl = mask_sb[:, ti, j * 8 : (j + 1) * 8]
                            mb = AP(
                                tensor=msl.tensor,
                                offset=msl.offset,
                                ap=list(msl.ap) + [[0, BLK]],
                            )
                            p3 = p_sb[:].rearrange("p (b w) -> p b w", w=BLK)
                            nc.vector.tensor_tensor(p3, p3, mb, op=OP.mult)
                            nc.tensor.matmul(
                                pv_ps,
                                v_sb[:, ti, :],
                                p_sb[:],
                                start=(ti == 0),
                                stop=(ti == ntile - 1),
                                skip_group_check=True,
                            )
                        sr = sm.tile([1, 512], F32)
                        nc.scalar.copy(sr[:], pv_ps[D : D + 1, :])
                        rc = sm.tile([1, 512], F32)
                        nc.vector.reciprocal(rc[:], sr[:])
                        rcb = sm.tile([1, 512], BF)
                        nc.vector.tensor_copy(rcb[:], rc[:])
                        rb_ps = rbs.tile([D, 512], F32)
                        nc.tensor.matmul(
                            rb_ps, ones_sb[:, :D], rcb[:], start=True, stop=True
                        )
                        # HW: DVE may read only ONE input from PSUM
                        rb_sb = sm.tile([D, 512], F32)
                        nc.scalar.copy(rb_sb[:], rb_ps[:])
                        nc.vector.tensor_mul(
                            attn_sb[:, hh, ssl], pv_ps[:D, :], rb_sb[:]
                        )

            # ---- phase 5: o-projection partial ----
            with tc.tile_pool(name="op_", bufs=4) as op_, tc.tile_pool(
                name="ops", bufs=4, space="PSUM"
            ) as ops:
                for si in range(NT):
                    tsl = slice(si * 128, (si + 1) * 128)
                    for ej in range(NE):
                        esl = slice(ej * 512, (ej + 1) * 512)
                        o_ps = ops.tile([128, 512], F32)
                        for hh in range(G):
                            nc.tensor.matmul(
                                o_ps,
                                attn_sb[:, hh, tsl],
                                ow_sb[:, hh, esl],
                                start=(hh == 0),
                                stop=(hh == G - 1),
                            )
                        o_sb = op_.tile([128, 512], BF)
                        nc.scalar.copy(o_sb[:], o_ps[:])
                        nc.sync.dma_start(out_d[tsl, esl], o_sb[:])

            if debug:
                for nm, t in [
                    ("dq", q_sb),
                    ("dk", k_sb),
                    ("dv", v_sb),
                    ("dmask", mask_sb),
                    ("dqp", qp_sb),
                    ("dkm", km_sb),
                    ("dkx", kx_sb),
                    ("dattn", attn_sb),
                ]:
                    dd = nc.dram_tensor(
                        nm, list(t[:].shape), t[:].dtype, kind="ExternalOutput"
                    )
                    nc.sync.dma_start(dd[:], t[:])
    return nc


def _host_prep(hidden_states, cos, sin, qkv_w, o_w, gate_wq, gate_wk):
    bf = ml_dtypes.bfloat16
    X = np.asarray(hidden_states, np.float32).reshape(S, HIDDEN)
    qkv_w = np.asarray(qkv_w, np.float32)
    o_w = np.asarray(o_w, np.float32)
    cos = np.asarray(cos, np.float32)
    sin = np.asarray(sin, np.float32)

    xt = np.ascontiguousarray(X.T).astype(bf)
    scale = D**-0.5
    cosT = np.ascontiguousarray(cos.T)
    sinT = np.ascontiguousarray(sin.T)
    cosq = (cosT * scale).astype(bf)
    sinq = (sinT * scale).astype(bf)
    cosk = cosT.astype(bf)
    sink = sinT.astype(bf)

    rt = np.zeros((D, D), np.float32)
    h = D // 2
    rt[np.arange(h) + h, np.arange(h)] = -1.0
    rt[np.arange(h), np.arange(h) + h] = 1.0
    rt = rt.astype(bf)

    emat = np.zeros((NB, NT * 128), np.float32)
    for i in range(NT):
        for p in range(128):
            emat[2 * i + p // BLK, i * 128 + p] = 1.0
    eye = np.eye(NB, dtype=np.float32)

    bcm = np.where(
        np.arange(NB)[None, :] <= np.arange(NB)[:, None], 0.0, -60.0
    ).astype(np.float32)
    # cmask[p, r*512+col] = 1 if col - p >= 128*r (k token ti*128+p causal
    # w.r.t. q token j*512+col on diagonal tiles, r = ti - 4j)
    p_i = np.arange(128)[:, None]
    cmask = np.zeros((128, 4 * 512), np.float32)
    for r in range(4):
        col = np.arange(512)[None, :]
        cmask[:, r * 512 : (r + 1) * 512] = (col - p_i >= 128 * r).astype(
            np.float32
        )
    cmask = cmask.astype(bf)

    # k block mean is computed on-device as a SUM; fold 1/BLK into the
    # mean-pool half of gate_wk
    gwk_s = np.asarray(gate_wk, np.float32).copy()
    gwk_s[:D, :] *= 1.0 / BLK

    common = dict(
        xt=xt,
        cosq=cosq,
        sinq=sinq,
        cosk=cosk,
        sink=sink,
        rot=rt,
        gwq=np.asarray(gate_wq, np.float32),
        gwk=gwk_s,
        eye32=eye,
        emat=emat,
        bcm=bcm,
        cmask=cmask,
    )
    maps = []
    for c in range(NCORES):
        maps.append(
            dict(
                common,
                wq=qkv_w[:, c * G * D : (c + 1) * G * D].astype(bf),
                wk=qkv_w[:, H * D + c * D : H * D + (c + 1) * D].astype(bf),
                wv=qkv_w[
                    :, H * D + HK * D + c * D : H * D + HK * D + (c + 1) * D
                ].astype(bf),
                ow=o_w[c * G * D : (c + 1) * G * D, :].astype(bf),
            )
        )
    return maps


def _gather(results):
    acc = np.zeros((S, HIDDEN), np.float32)
    for r in results:
        acc += np.asarray(r["out_p"]).astype(np.float32)
    return acc.reshape(1, S, HIDDEN)


def _run(inputs, trace=False):
    global _prog
    if _prog is None:
        _prog = _build()
        if not _prog.is_finalized():
            _prog.finalize()
    from concourse import bass_utils

    maps = _host_prep(**inputs)
    res = bass_utils.run_bass_kernel_spmd(
        _prog, maps, list(range(NCORES)), trace=trace
    )
    return _gather(res.results), res


def kernel(**inputs):
    out, _ = _run(inputs, trace=False)
    return out



# revision 8
# speedup vs baseline: 1.4991x; 1.4991x over previous
import sys

sys.path.insert(0, "/opt/trn_rl_repo")

import numpy as np
import ml_dtypes

# Phi3SeerAttention, B=1 S=2048 HIDDEN=3072, H=32 q heads, HK=8 kv heads,
# D=96, gate block 64, gate hidden 128. Sharded TP over kv heads: core c
# owns kv head c and q heads 4c..4c+3; o-proj row-sharded, partials summed
# on host (the gather step).
#
# Single TileContext, chunk-pipelined (j = 512-token chunk sweep):
#   per chunk: QKV (packed-128 column groups) -> repartition -> gate pooling
#   -> RoPE -> block gate -> masked attention (causal-trimmed) -> o-proj.
# Engine split: PE matmuls; ACT exp; DVE masks/rope/reductions; Pool copies.
H, HK, D, BLK, GH = 32, 8, 96, 64, 128
S, HIDDEN = 2048, 3072
G = H // HK          # 4 q heads per kv head (per core)
NB = S // BLK        # 32 gate blocks
KT = HIDDEN // 128   # 24 contraction tiles
NS = S // 512        # 4 sequence chunks of 512
NT = S // 128        # 16 k-tiles of 128
NE = HIDDEN // 512   # 6 output column chunks
NCORES = 8
THR = 0.03
SCALE = float(D) ** -0.5
GSCALE = (1.0 / (G * BLK)) * float(GH) ** -0.5

_prog = None


def _build(debug=False):
    from concourse import bass, mybir, bacc
    import concourse.tile as tile
    from concourse.bass import AP
    from contextlib import ExitStack

    dt = mybir.dt
    BF, F32 = dt.bfloat16, dt.float32
    AF = mybir.ActivationFunctionType
    OP = mybir.AluOpType
    AX = mybir.AxisListType.X

    nc = bacc.Bacc()
    xt_d = nc.dram_tensor("xt", [HIDDEN, S], BF, kind="ExternalInput")
    wqkv_d = nc.dram_tensor("wqkv", [HIDDEN, 6 * D], BF, kind="ExternalInput")
    ow_d = nc.dram_tensor("ow", [G * D, HIDDEN], BF, kind="ExternalInput")
    cos_d = nc.dram_tensor("cos", [D, S], BF, kind="ExternalInput")
    sinn_d = nc.dram_tensor("sinn", [D, S], BF, kind="ExternalInput")
    gwq_d = nc.dram_tensor("gwq", [D, GH], F32, kind="ExternalInput")
    gwk2_d = nc.dram_tensor("gwk2", [D, 2, GH], F32, kind="ExternalInput")
    bcm4_d = nc.dram_tensor("bcm4", [8, NS, NB], F32, kind="ExternalInput")
    eye4_d = nc.dram_tensor("eye4", [8, NS, NB], F32, kind="ExternalInput")
    id8_d = nc.dram_tensor("id8", [8, 8], F32, kind="ExternalInput")
    emat_d = nc.dram_tensor("emat", [NB, NT * 128], F32, kind="ExternalInput")
    cmask_d = nc.dram_tensor("cmask", [128, 4 * 512], BF, kind="ExternalInput")
    out_d = nc.dram_tensor("out_p", [S, HIDDEN], BF, kind="ExternalOutput")

    with tile.TileContext(nc) as tc:
        with ExitStack() as ctx:
            perm = ctx.enter_context(tc.tile_pool(name="perm", bufs=1))
            # ---- persistent SBUF ----
            w_sb = perm.tile([128, KT, 6 * D], BF, name="w_sb")
            q_sb = perm.tile([D, G, S], BF, name="q_sb")
            k_sb = perm.tile([D, S], BF, name="k_sb")
            v_sb = perm.tile([128, NT, D + 1], BF, name="v_sb")
            attn_sb = perm.tile([128, 3, S], BF, name="attn_sb")
            owp_sb = perm.tile([128, 3, HIDDEN], BF, name="owp_sb")
            cos_sb = perm.tile([D, S], BF, name="cos_sb")
            sinn_sb = perm.tile([D, S], BF, name="sinn_sb")
            cmask_sb = perm.tile([128, 4 * 512], BF, name="cmask_sb")
            emat_sb = perm.tile([NB, NT * 128], F32, name="emat_sb")
            gwq_sb = perm.tile([D, GH], F32, name="gwq_sb")
            gwk_sb = perm.tile([D, 2, GH], F32, name="gwk_sb")
            bcm4_sb = perm.tile([8, NS, NB], F32, name="bcm4_sb")
            eye4_sb = perm.tile([8, NS, NB], F32, name="eye4_sb")
            id8_sb = perm.tile([8, 8], F32, name="id8_sb")
            mask_sb = perm.tile([128, NT, 8], BF, name="mask_sb")
            mexd_sb = perm.tile([128, 4, 512], BF, name="mexd_sb")
            ones_sb = perm.tile([1, D], BF, name="ones_sb")
            qp_sb = perm.tile([D, NB], F32, name="qp_sb")
            km_sb = perm.tile([D, NB], F32, name="km_sb")
            kx_sb = perm.tile([D, NB], F32, name="kx_sb")
            kgT_sb = perm.tile([GH, NB], F32, name="kgT_sb")
            qgT_sb = perm.tile([GH, 8], F32, name="qgT_sb")
            lgm_sb = perm.tile([8, NB], F32, name="lgm_sb")
            ge_sb = perm.tile([8, NB], F32, name="ge_sb")
            gsum_sb = perm.tile([8, 1], F32, name="gsum_sb")
            grc_sb = perm.tile([8, 1], F32, name="grc_sb")
            m01_sb = perm.tile([8, NB], F32, name="m01_sb")
            m01t_sb = perm.tile([NB, 8], F32, name="m01t_sb")
            thr_sb = perm.tile([8, NB], F32, name="thr_sb")

            # ---- rotating pools ----
            xtp = ctx.enter_context(tc.tile_pool(name="xtp", bufs=2))
            qgp_ = ctx.enter_context(tc.tile_pool(name="qgp_", bufs=2))
            rnp = ctx.enter_context(tc.tile_pool(name="rnp", bufs=2))
            ropep = ctx.enter_context(tc.tile_pool(name="ropep", bufs=2))
            qsump = ctx.enter_context(tc.tile_pool(name="qsump", bufs=2))
            psb = ctx.enter_context(tc.tile_pool(name="psb", bufs=4))
            rbp = ctx.enter_context(tc.tile_pool(name="rbp", bufs=2))
            rcp = ctx.enter_context(tc.tile_pool(name="rcp", bufs=2))
            atp = ctx.enter_context(tc.tile_pool(name="atp", bufs=3))
            outp = ctx.enter_context(tc.tile_pool(name="outp", bufs=2))
            qkvps = ctx.enter_context(
                tc.tile_pool(name="qkvps", bufs=2, space="PSUM")
            )
            scps = ctx.enter_context(
                tc.tile_pool(name="scps", bufs=2, space="PSUM")
            )
            pvps = ctx.enter_context(
                tc.tile_pool(name="pvps", bufs=2, space="PSUM")
            )
            opps = ctx.enter_context(
                tc.tile_pool(name="opps", bufs=2, space="PSUM")
            )

            # ---- preload DMAs ----
            for wi in range(4):
                r = slice(wi * 6 * 128, (wi + 1) * 6 * 128)
                src = wqkv_d[r, :].rearrange("(k p) c -> p k c", p=128)
                nc.sync.dma_start(w_sb[:, wi * 6 : (wi + 1) * 6, :], src)
            nc.sync.dma_start(
                owp_sb[:], ow_d[:].rearrange("(t p) e -> p t e", p=128)
            )
            nc.sync.dma_start(cos_sb[:], cos_d[:])
            nc.sync.dma_start(sinn_sb[:], sinn_d[:])
            nc.sync.dma_start(cmask_sb[:], cmask_d[:])
            nc.sync.dma_start(emat_sb[:], emat_d[:])
            nc.sync.dma_start(gwq_sb[:], gwq_d[:])
            nc.sync.dma_start(gwk_sb[:], gwk2_d[:])
            nc.sync.dma_start(bcm4_sb[:], bcm4_d[:])
            nc.sync.dma_start(eye4_sb[:], eye4_d[:])
            nc.sync.dma_start(id8_sb[:], id8_d[:])
            nc.vector.memset(ones_sb[:], 1.0)
            nc.vector.memset(v_sb[:, :, D : D + 1], 1.0)
            nc.vector.memset(thr_sb[:], THR)
            nc.vector.memset(m01t_sb[:], 0.0)

            for j in range(NS):
                sl = slice(j * 512, (j + 1) * 512)
                bs = slice(j * 8, (j + 1) * 8)
                ntile = 4 * (j + 1)

                # ---- QKV projection for chunk j ----
                xt_t = xtp.tile([128, KT, 512], BF, name=f"xt{j}", tag="xt")
                if j == 0:
                    # split first chunk's load so kt-0 matmuls start early
                    for wi in range(4):
                        r = slice(wi * 6 * 128, (wi + 1) * 6 * 128)
                        src = xt_d[r, sl].rearrange("(k p) s -> p k s", p=128)
                        nc.sync.dma_start(xt_t[:, wi * 6 : (wi + 1) * 6, :], src)
                else:
                    src = xt_d[:, sl].rearrange("(k p) s -> p k s", p=128)
                    nc.sync.dma_start(xt_t[:], src)
                # packed q groups: wqkv cols [0:128),[128:256),[256:384)
                gps = []
                for g in range(3):
                    ps = qkvps.tile([128, 512], F32, name=f"qg{j}_{g}", tag="qk")
                    for kt in range(KT):
                        nc.tensor.matmul(
                            ps,
                            w_sb[:, kt, g * 128 : (g + 1) * 128],
                            xt_t[:, kt, :],
                            start=(kt == 0),
                            stop=(kt == KT - 1),
                            skip_group_check=True,
                        )
                    gps.append(ps)
                # k group: cols [384:480)
                kps = qkvps.tile([D, 512], F32, name=f"kg{j}", tag="qk")
                for kt in range(KT):
                    nc.tensor.matmul(
                        kps,
                        w_sb[:, kt, 4 * D : 5 * D],
                        xt_t[:, kt, :],
                        start=(kt == 0),
                        stop=(kt == KT - 1),
                        skip_group_check=True,
                    )
                # repartition: ACT copies psum groups to SBUF, then
                # partition-shifting DMAs scatter into per-head q_sb.
                qg_sb = qgp_.tile([128, 3, 512], BF, name=f"qg{j}", tag="qg")
                for g in range(3):
                    nc.scalar.copy(qg_sb[:, g, :], gps[g][:])
                nc.scalar.copy(k_sb[:, sl], kps[:])
                # g0: p<96 -> q0 d=p ; p>=96 -> q1 d=p-96
                nc.sync.dma_start(q_sb[0:96, 0, sl], qg_sb[0:96, 0, :])
                nc.sync.dma_start(q_sb[0:32, 1, sl], qg_sb[96:128, 0, :])
                # g1: p<64 -> q1 d=32+p ; p>=64 -> q2 d=p-64
                nc.sync.dma_start(q_sb[32:96, 1, sl], qg_sb[0:64, 1, :])
                nc.sync.dma_start(q_sb[0:64, 2, sl], qg_sb[64:128, 1, :])
                # g2: p<32 -> q2 d=64+p ; p>=32 -> q3 d=p-32
                nc.sync.dma_start(q_sb[64:96, 2, sl], qg_sb[0:32, 2, :])
                nc.sync.dma_start(q_sb[0:96, 3, sl], qg_sb[32:128, 2, :])
                # v token-major: for each 128-token tile
                for ti in range(4 * j, 4 * (j + 1)):
                    vps = qkvps.tile([128, D], F32, name=f"v{j}_{ti}", tag="qk")
                    for kt in range(KT):
                        nc.tensor.matmul(
                            vps,
                            xt_t[:, kt, (ti - 4 * j) * 128 : (ti - 4 * j + 1) * 128],
                            w_sb[:, kt, 5 * D : 6 * D],
                            start=(kt == 0),
                            stop=(kt == KT - 1),
                            skip_group_check=True,
                        )
                    nc.scalar.copy(v_sb[:, ti, :D], vps[:])

                # ---- gate pooling (pre-RoPE) ----
                qs1 = qsump.tile([D, 512], BF, name=f"qs1_{j}", tag="qs1")
                qs2 = qsump.tile([D, 512], BF, name=f"qs2_{j}", tag="qs2")
                nc.gpsimd.tensor_add(qs1[:], q_sb[:, 0, sl], q_sb[:, 1, sl])
                nc.gpsimd.tensor_add(qs2[:], q_sb[:, 2, sl], q_sb[:, 3, sl])
                nc.gpsimd.tensor_add(qs1[:], qs1[:], qs2[:])
                nc.vector.tensor_reduce(
                    qp_sb[:, bs],
                    qs1[:].rearrange("p (b w) -> p b w", w=BLK),
                    axis=AX,
                    op=OP.add,
                )
                kv = k_sb[:, sl].rearrange("p (b w) -> p b w", w=BLK)
                nc.vector.tensor_reduce(km_sb[:, bs], kv, axis=AX, op=OP.add)
                nc.vector.tensor_reduce(kx_sb[:, bs], kv, axis=AX, op=OP.max)

                # ---- RoPE in place on q^T / k^T ----
                # rotate-half = partition shift by 48 (sign folded into sinn)
                hD = D // 2
                rn = rnp.tile([D, G + 1, 512], BF, name=f"rn{j}", tag="rn")
                nc.sync.dma_start(rn[0:hD, 0:G, :], q_sb[hD:D, :, sl])
                nc.sync.dma_start(rn[hD:D, 0:G, :], q_sb[0:hD, :, sl])
                nc.sync.dma_start(rn[0:hD, G, :], k_sb[hD:D, sl])
                nc.sync.dma_start(rn[hD:D, G, :], k_sb[0:hD, sl])
                for hh in range(G + 1):
                    src = q_sb[:, hh, sl] if hh < G else k_sb[:, sl]
                    t1 = ropep.tile([D, 512], BF, name=f"t1_{j}_{hh}", tag="t1")
                    t2 = ropep.tile([D, 512], BF, name=f"t2_{j}_{hh}", tag="t2")
                    nc.gpsimd.tensor_mul(t1[:], src, cos_sb[:, sl])
                    nc.gpsimd.tensor_mul(t2[:], rn[:, hh, :], sinn_sb[:, sl])
                    nc.gpsimd.tensor_add(src, t1[:], t2[:])

                # ---- block gate for chunk j's 8 q-blocks ----
                kb = 8 * (j + 1)
                kgp = qkvps.tile([GH, 8], F32, name=f"kgp{j}", tag="qk")
                nc.tensor.matmul(
                    kgp, gwk_sb[:, 0, :], km_sb[:, bs], start=True, stop=False
                )
                nc.tensor.matmul(
                    kgp, gwk_sb[:, 1, :], kx_sb[:, bs], start=False, stop=True
                )
                nc.vector.tensor_copy(kgT_sb[:, bs], kgp[:])
                qgp = qkvps.tile([GH, 8], F32, name=f"qgp{j}", tag="qk")
                nc.tensor.matmul(qgp, gwq_sb[:], qp_sb[:, bs], start=True, stop=True)
                nc.scalar.mul(qgT_sb[:], qgp[:], GSCALE)
                lgp = qkvps.tile([8, NB], F32, name=f"lgp{j}", tag="qk")
                nc.tensor.matmul(
                    lgp[:, :kb], qgT_sb[:], kgT_sb[:, :kb], start=True, stop=True
                )
                nc.vector.tensor_add(
                    lgm_sb[:, :kb], lgp[:, :kb], bcm4_sb[:, j, :kb]
                )
                nc.scalar.activation(
                    ge_sb[:, :kb], lgm_sb[:, :kb], AF.Exp, accum_out=gsum_sb[:]
                )
                nc.vector.reciprocal(grc_sb[:], gsum_sb[:])
                nc.vector.scalar_tensor_tensor(
                    m01_sb[:, :kb],
                    ge_sb[:, :kb],
                    grc_sb[:],
                    thr_sb[:, :kb],
                    op0=OP.mult,
                    op1=OP.is_ge,
                )
                nc.vector.tensor_tensor(
                    m01_sb[:, :kb], m01_sb[:, :kb], eye4_sb[:, j, :kb], op=OP.max
                )
                m01tp = qkvps.tile([NB, 8], F32, name=f"m01tp{j}", tag="qk")
                nc.tensor.transpose(m01tp[:kb, :], m01_sb[:, :kb], id8_sb[:])
                nc.vector.tensor_copy(m01t_sb[:kb, :], m01tp[:kb, :])
                # expand k-blocks to token rows: mask_sb[:, ti, qb]
                for ti in range(ntile):
                    mp = qkvps.tile([128, 8], F32, name=f"mp{j}_{ti}", tag="qk")
                    nc.tensor.matmul(
                        mp,
                        emat_sb[:, ti * 128 : (ti + 1) * 128],
                        m01t_sb[:],
                        start=True,
                        stop=True,
                    )
                    nc.vector.tensor_copy(mask_sb[:, ti, :], mp[:])
                # diagonal tiles: fold block mask into the causal cmask
                for r in range(4):
                    ti = 4 * j + r
                    c0 = 128 * r
                    msl = mask_sb[:, ti, c0 // BLK : 8]
                    mb = AP(
                        tensor=msl.tensor,
                        offset=msl.offset,
                        ap=list(msl.ap) + [[0, BLK]],
                    )
                    dst = mexd_sb[:, r, c0:].rearrange("p (b w) -> p b w", w=BLK)
                    cm = cmask_sb[:, r * 512 + c0 : (r + 1) * 512].rearrange(
                        "p (b w) -> p b w", w=BLK
                    )
                    nc.gpsimd.tensor_tensor(dst, cm, mb, op=OP.mult)

                # ---- masked attention for chunk j ----
                for hh in range(G):
                    pv = pvps.tile([D + 1, 512], F32, name=f"pv{j}_{hh}", tag="pv")
                    for ti in range(ntile):
                        r = ti - 4 * j
                        c0 = 128 * r if r >= 0 else 0
                        sp = scps.tile([128, 512], F32, name=f"s{j}_{hh}_{ti}", tag="s")
                        nc.tensor.matmul(
                            sp[:, c0:],
                            k_sb[:, ti * 128 : (ti + 1) * 128],
                            q_sb[:, hh, j * 512 + c0 : (j + 1) * 512],
                            start=True,
                            stop=True,
                            skip_group_check=True,
                        )
                        pt = psb.tile([128, 512], BF, name=f"p{j}_{hh}_{ti}", tag="p")
                        nc.scalar.activation(
                            pt[:, c0:], sp[:, c0:], AF.Exp, scale=SCALE
                        )
                        if r >= 0:
                            nc.gpsimd.tensor_mul(
                                pt[:, c0:], pt[:, c0:], mexd_sb[:, r, c0:]
                            )
                        else:
                            msl = mask_sb[:, ti, :]
                            mb = AP(
                                tensor=msl.tensor,
                                offset=msl.offset,
                                ap=list(msl.ap) + [[0, BLK]],
                            )
                            nc.gpsimd.tensor_tensor(
                                pt[:].rearrange("p (b w) -> p b w", w=BLK),
                                pt[:].rearrange("p (b w) -> p b w", w=BLK),
                                mb,
                                op=OP.mult,
                            )
                        nc.tensor.matmul(
                            pv[:, c0:],
                            v_sb[:, ti, :],
                            pt[:, c0:],
                            start=(ti == 0),
                            stop=(ti == ntile - 1),
                            skip_group_check=True,
                        )
                    rcb = rcp.tile([1, 512], BF, name=f"rc{j}_{hh}", tag="rc")
                    with nc.allow_low_precision(reason="softmax recip to bf16"):
                        nc.vector.reciprocal(rcb[:], pv[D : D + 1, :])
                    rb_ps = opps.tile([D, 512], F32, name=f"rb{j}_{hh}", tag="o")
                    nc.tensor.matmul(rb_ps, ones_sb[:], rcb[:], start=True, stop=True)
                    rb_sb = rbp.tile([D, 512], BF, name=f"rbs{j}_{hh}", tag="rb")
                    nc.vector.tensor_copy(rb_sb[:], rb_ps[:])
                    if hh == 0:
                        nc.vector.tensor_mul(
                            attn_sb[0:96, 0, sl], pv[:D, :], rb_sb[:]
                        )
                    else:
                        at = atp.tile([D, 512], BF, name=f"at{j}_{hh}", tag="at")
                        nc.vector.tensor_mul(at[:], pv[:D, :], rb_sb[:])
                        if hh == 1:
                            nc.sync.dma_start(attn_sb[96:128, 0, sl], at[0:32, :])
                            nc.sync.dma_start(attn_sb[0:64, 1, sl], at[32:96, :])
                        elif hh == 2:
                            nc.sync.dma_start(attn_sb[64:128, 1, sl], at[0:64, :])
                            nc.sync.dma_start(attn_sb[0:32, 2, sl], at[64:96, :])
                        else:
                            nc.sync.dma_start(attn_sb[32:128, 2, sl], at[:])

                # ---- o-projection for chunk j's token tiles ----
                for si in range(4 * j, 4 * (j + 1)):
                    tsl = slice(si * 128, (si + 1) * 128)
                    ot = outp.tile([128, NE, 512], BF, name=f"ot{si}", tag="ot")
                    for ej in range(NE):
                        esl = slice(ej * 512, (ej + 1) * 512)
                        ops_ = opps.tile([128, 512], F32, name=f"o{si}_{ej}", tag="o")
                        for t in range(3):
                            nc.tensor.matmul(
                                ops_,
                                attn_sb[:, t, tsl],
                                owp_sb[:, t, esl],
                                start=(t == 0),
                                stop=(t == 2),
                                skip_group_check=True,
                            )
                        if ej < 5:
                            nc.vector.tensor_copy(ot[:, ej, :], ops_[:])
                        else:
                            nc.scalar.copy(ot[:, ej, :], ops_[:])
                    nc.sync.dma_start(out_d[tsl, :], ot[:])
    return nc


def _host_prep(hidden_states, cos, sin, qkv_w, o_w, gate_wq, gate_wk):
    bf = ml_dtypes.bfloat16
    X = np.asarray(hidden_states, np.float32).reshape(S, HIDDEN)
    qkv_w = np.asarray(qkv_w, np.float32)
    o_w = np.asarray(o_w, np.float32)
    cos = np.asarray(cos, np.float32)
    sin = np.asarray(sin, np.float32)

    xt = np.ascontiguousarray(X.T).astype(bf)
    cosT = np.ascontiguousarray(cos.T).astype(bf)
    # rotate-half sign is folded into the sin table: rows 0:48 negated
    sinn = np.ascontiguousarray(sin.T).copy()
    sinn[: D // 2, :] *= -1.0
    sinn = sinn.astype(bf)

    emat = np.zeros((NB, NT * 128), np.float32)
    for i in range(NT):
        for p in range(128):
            emat[2 * i + p // BLK, i * 128 + p] = 1.0

    # cmask[p, r*512+col] = 1 if col - p >= 128*r (token causality on the
    # r-th diagonal k-tile of a 512-token q chunk)
    p_i = np.arange(128)[:, None]
    cmask = np.zeros((128, 4 * 512), np.float32)
    for r in range(4):
        col = np.arange(512)[None, :]
        cmask[:, r * 512 : (r + 1) * 512] = (col - p_i >= 128 * r).astype(
            np.float32
        )
    cmask = cmask.astype(bf)

    # block-causal bias and forced-diagonal for the gate, laid out per chunk:
    # row r, chunk jj -> q block 8*jj + r
    bcm4 = np.zeros((8, NS, NB), np.float32)
    eye4 = np.zeros((8, NS, NB), np.float32)
    for jj in range(NS):
        for r in range(8):
            qb = 8 * jj + r
            bcm4[r, jj, :] = np.where(np.arange(NB) <= qb, 0.0, -60.0)
            eye4[r, jj, qb] = 1.0
    id8 = np.eye(8, dtype=np.float32)

    # k block mean is computed on-device as a SUM; fold 1/BLK into the
    # mean-pool half of gate_wk. gwk2[d, 0, :] = mean part, [d, 1, :] = max.
    gwk_s = np.asarray(gate_wk, np.float32).copy()
    gwk_s[:D, :] *= 1.0 / BLK
    gwk2 = np.stack([gwk_s[:D, :], gwk_s[D:, :]], axis=1)

    common = dict(
        xt=xt,
        cos=cosT,
        sinn=sinn,
        gwq=np.asarray(gate_wq, np.float32),
        gwk2=np.ascontiguousarray(gwk2),
        bcm4=bcm4,
        eye4=eye4,
        id8=id8,
        emat=emat,
        cmask=cmask,
    )
    maps = []
    for c in range(NCORES):
        wq = qkv_w[:, c * G * D : (c + 1) * G * D]
        wk = qkv_w[:, H * D + c * D : H * D + (c + 1) * D]
        wv = qkv_w[:, H * D + HK * D + c * D : H * D + HK * D + (c + 1) * D]
        wqkv = np.concatenate([wq, wk, wv], axis=1).astype(bf)
        maps.append(
            dict(
                common,
                wqkv=np.ascontiguousarray(wqkv),
                ow=o_w[c * G * D : (c + 1) * G * D, :].astype(bf),
            )
        )
    return maps


def _gather(results):
    acc = np.zeros((S, HIDDEN), np.float32)
    for r in results:
        acc += np.asarray(r["out_p"]).astype(np.float32)
    return acc.reshape(1, S, HIDDEN)


def _run(inputs, trace=False):
    global _prog
    if _prog is None:
        _prog = _build()
        if not _prog.is_finalized():
            _prog.finalize()
    from concourse import bass_utils

    maps = _host_prep(**inputs)
    res = bass_utils.run_bass_kernel_spmd(
        _prog, maps, list(range(NCORES)), trace=trace
    )
    return _gather(res.results), res


def kernel(**inputs):
    out, _ = _run(inputs, trace=False)
    return out
